# revision 1
# baseline (speedup 1.0000x reference)
"""Trainium2 Bass kernel for Physics-Attention over an irregular mesh.

Contract: kernel(**inputs) takes the FULL inputs from setup_inputs() and
returns the FULL [4, 32768, 256] f32 output, distributing across 8 cores
internally (one (batch, half-of-N) shard per core; the slice-token pooling
reductions are exchanged with a pairwise AllGather + local add).

Structure per core (16384 tokens):
  pass 1 (128 token-tiles): logits/features matmuls (fp8 DoubleRow),
    softmax over slices, pooling into slice tokens (PSUM-resident
    accumulators), PE transposes of the routing weights stored for pass 2.
  exchange: two staggered AllGathers (tiles 0..63 / 64..127) so the first
    overlaps the second half of pass 1.
  stage: tiny cross-attention among 64 slice tokens.
  pass 2: out^T = C^T-stationary matmuls over the stored routing weights,
    written transposed and fixed up on the host.
"""

import sys

sys.path.insert(0, "/opt/trn_rl_repo")

import numpy as np
import ml_dtypes

import concourse.bass as bass
import concourse.mybir as mybir
import concourse.tile as tile
from concourse import bacc, bass_utils
from concourse.bass import ts

F32 = mybir.dt.float32
BF16 = mybir.dt.bfloat16
FP8 = mybir.dt.float8e4
NP_FP8 = ml_dtypes.float8_e4m3
AF = mybir.ActivationFunctionType
ALU = mybir.AluOpType
DR = mybir.MatmulPerfMode.DoubleRow

B, N, DIM = 4, 32768, 256
H, D, G = 8, 64, 64
INNER = H * D  # 512
NCORES = 8
NLOC = N // 2          # 16384 tokens per core
TOK = 128              # tokens per tile
T = NLOC // TOK        # 128 tiles
KCH = DIM // 128       # 2 contraction chunks
EPS_SLICE = 1e-5

# precision knobs (validated against the 2e-2 rel-err budget)
FP8_FX = False          # x@Wfx (features) in fp8 DoubleRow; logits stay bf16
FP8_P2 = False         # pass-2 swT @ C in fp8 DoubleRow
WSCALE = 8.0           # fp8 weight pre-scale (folded back out downstream)
SPLIT_COLL = True     # stagger the pooling exchange in two halves

_CACHE = {}


def _build(attn_scale: float, res_scale: float):
    nc = bacc.Bacc("TRN2", target_bir_lowering=False, debug=False,
                   enable_asserts=False, num_devices=NCORES)

    fxdt = FP8 if FP8_FX else BF16
    xT_d = nc.dram_tensor("xT", [DIM, NLOC], BF16, kind="ExternalInput").ap()
    AT_d = nc.dram_tensor("AT", [DIM, INNER], BF16, kind="ExternalInput").ap()
    WfxT_d = nc.dram_tensor("WfxT", [DIM, INNER], fxdt, kind="ExternalInput").ap()
    idbf_d = nc.dram_tensor("idbf", [128, 128], BF16, kind="ExternalInput").ap()
    id32_d = nc.dram_tensor("id32", [64, 64], F32, kind="ExternalInput").ap()
    WqT_d = nc.dram_tensor("WqT", [2 * D, D], F32, kind="ExternalInput").ap()
    WkT_d = nc.dram_tensor("WkT", [D, D], F32, kind="ExternalInput").ap()
    WvT_d = nc.dram_tensor("WvT", [D, D], F32, kind="ExternalInput").ap()
    WoT_d = nc.dram_tensor("WoT", [128, H, DIM], BF16, kind="ExternalInput").ap()
    # transposed output: [f-block, f-part, token]
    outT_d = nc.dram_tensor("outT", [2, 128, NLOC], F32, kind="ExternalOutput").ap()

    xT_v = xT_d.rearrange("(c p) n -> p c n", p=128)    # [128, 2, NLOC]
    AT_v = AT_d.rearrange("(c p) n -> p c n", p=128)    # [128, 2, 512]
    WfxT_v = WfxT_d.rearrange("(c p) n -> p c n", p=128)
    WoT_v = WoT_d  # [128, 8, 256], both 64-row halves identical

    p2dt = FP8 if FP8_P2 else BF16

    with tile.TileContext(nc) as tc:
        with (
            tc.tile_pool(name="consts", bufs=1) as consts,
            tc.tile_pool(name="store", bufs=1) as store,
            tc.tile_pool(name="work", bufs=3) as work,
            tc.tile_pool(name="small", bufs=3) as small,
            tc.tile_pool(name="obuf", bufs=2) as obuf,
            tc.tile_pool(name="stage", bufs=1) as stg_pool,
            tc.tile_pool(name="psmm", bufs=2, space="PSUM") as psmm,
            tc.tile_pool(name="psacc", bufs=1, space="PSUM") as psacc,
            tc.tile_pool(name="dram", bufs=1, space="DRAM") as dram,
        ):
            # ---- resident constants ----
            AT_sb = consts.tile([128, KCH, INNER], BF16)
            nc.sync.dma_start(AT_sb, AT_v)
            WfxT_sb = consts.tile([128, KCH, INNER], fxdt)
            nc.sync.dma_start(WfxT_sb, WfxT_v)
            idbf = consts.tile([128, 128], BF16)
            nc.sync.dma_start(idbf, idbf_d)
            id32 = consts.tile([64, 64], F32)
            nc.sync.dma_start(id32, id32_d)
            WqT2_sb = consts.tile([128, 64], F32)
            nc.sync.dma_start(WqT2_sb, WqT_d)
            WkT_sb = consts.tile([64, 64], F32)
            nc.sync.dma_start(WkT_sb, WkT_d)
            WvT_sb = consts.tile([64, 64], F32)
            nc.sync.dma_start(WvT_sb, WvT_d)
            WoT2_sb = consts.tile([128, H, DIM], BF16)
            nc.sync.dma_start(WoT2_sb, WoT_v)

            # transposed routing weights, chunk-major over contiguous tokens:
            # [128 (hg in chunk), 4 chunks, NLOC]
            swT_store = store.tile([128, 4, NLOC], p2dt)
            # slice-token accumulator: [p, hp, blk, 0:64]=st_un for head
            # h=2*blk+hp at partitions hp*64+g, col 64 = snorm. dim1 selects
            # a bank per partition-half so each half owns its zero region.
            st_ps = psacc.tile([128, 2, 4, 128], F32, name="st_ps")

            # exchange buffers (DRAM round-trip, proven HW path)
            cc_in = [dram.tile([128, 4 * (D + 1)], F32, name=f"cc_in{i}")
                     for i in range(2)]
            cc_out = [dram.tile([2, 128, 4 * (D + 1)], F32, name=f"cc_out{i}")
                      for i in range(2)]

            # preset the ones column of the rotating fxs buffers
            fxs_bufs = []
            for i in range(3):
                fxs = work.tile([128, 2, H, D + 1], BF16, tag="fxs")
                nc.gpsimd.memset(fxs[:, :, :, D], 1.0)
                fxs_bufs.append(fxs)

            # ================= PASS 1 =================
            # two token-tiles per iteration share one exp/reduce/recip/
            # normalize/feature-copy instruction (half the fixed overheads)
            stun = [None, None]
            for pr in range(T // 2):
                t0 = 2 * pr
                xt2 = work.tile([128, KCH, 2 * TOK], BF16, tag="xt2")
                nc.sync.dma_start(xt2, xT_v[:, :, t0 * TOK:(t0 + 2) * TOK])
                lg2 = psacc.tile([128, 2, H * G], F32, name="lg2")
                fx2 = psacc.tile([128, 2, H * D], F32, name="fx2")
                for i in range(2):
                    xt = xt2[:, :, i * TOK:(i + 1) * TOK]
                    for k in range(KCH):
                        nc.tensor.matmul(lg2[:, i, :], xt[:, k, :],
                                         AT_sb[:, k, :],
                                         start=(k == 0), stop=(k == KCH - 1))
                        nc.tensor.matmul(fx2[:, i, :], xt[:, k, :],
                                         WfxT_sb[:, k, :],
                                         start=(k == 0), stop=(k == KCH - 1))

                # softmax over slices (exp descales the fp8 weight pre-scale)
                usw = work.tile([128, 2, H, G], BF16, tag="usw")
                nc.scalar.activation(usw.rearrange("p i h g -> p i (h g)"),
                                     lg2, AF.Exp)
                den = small.tile([128, 2, H], F32, tag="den")
                nc.vector.reduce_sum(den, usw, axis=mybir.AxisListType.X)
                rden = small.tile([128, 2, H], F32, tag="rden")
                nc.vector.reciprocal(rden, den)
                swn = work.tile([128, 2, H, G], BF16, tag="swn")
                nc.gpsimd.tensor_tensor(
                    swn, usw, rden[:, :, :, None].to_broadcast([128, 2, H, G]),
                    ALU.mult)

                fxs = fxs_bufs[pr % 3]
                nc.scalar.copy(fxs[:, :, :, 0:D],
                               fx2.rearrange("p i (h d) -> p i h d", d=D))

                for i in range(2):
                    t = t0 + i
                    swn2 = swn[:, i].rearrange("p h g -> p (h g)")
                    swtp = psmm.tile([128, 4, TOK], BF16, tag="swtp")
                    if SPLIT_COLL:
                        first, last = t % 64 == 0, t % 64 == 63
                    else:
                        first, last = t == 0, t == T - 1
                    for blk in range(4):
                        nc.tensor.transpose(swtp[:, blk, :],
                                            swn2[:, ts(blk, 128)], idbf)
                        for hp in range(2):
                            lhs = swn2[:, blk * 128 + hp * 64:
                                       blk * 128 + (hp + 1) * 64]
                            nc.tensor.matmul(
                                st_ps[64 * hp:64 * (hp + 1), hp, blk, 0:D + 1],
                                lhs, fxs[:, i, 2 * blk + hp, :],
                                start=(first and blk == 0),
                                stop=(last and blk == 3))
                    nc.vector.tensor_copy(
                        swT_store[:, :, t * TOK:(t + 1) * TOK], swtp)

                    if t % 64 == 63 and (t == T - 1 or SPLIT_COLL):
                        half = t // 64
                        stun[half] = stg_pool.tile([128, 4, D + 1], F32,
                                                   name=f"stun{half}")
                        nc.vector.tensor_copy(stun[half][0:64],
                                              st_ps[0:64, 0, :, 0:D + 1])
                        nc.scalar.copy(stun[half][64:128],
                                       st_ps[64:128, 1, :, 0:D + 1])
                        nc.sync.dma_start(
                            cc_in[half],
                            stun[half].rearrange("p a b -> p (a b)"))
                        nc.gpsimd.collective_compute(
                            "AllGather", ALU.bypass,
                            replica_groups=[[0, 1], [2, 3], [4, 5], [6, 7]],
                            ins=[cc_in[half].opt()],
                            outs=[cc_out[half].rearrange(
                                "r p e -> (r p) e").opt()],
                        )

            # ============ STAGE (slice attention, tiny) ============
            # load reduced halves back with g on partitions:
            # cc_out[i] flat = [hp*64+g, blk, 65] -> [g, hp, blk, 65]
            gth = [None, None]
            for i in ([0, 1] if SPLIT_COLL else [1]):
                gth[i] = stg_pool.tile([64, 2, 2, 4, D + 1], F32,
                                       name=f"gth{i}")
                src = cc_out[i].rearrange("r (hp g) (blk e) -> g r hp blk e",
                                          hp=2, e=D + 1)
                nc.sync.dma_start(gth[i], src)
            # stg[g, hp, blk, e], heads h = 2*blk + hp
            stg = stg_pool.tile([64, 2, 4, D + 1], F32)
            s0 = stg_pool.tile([64, 2, 4, D + 1], F32, name="s0")
            if SPLIT_COLL:
                nc.vector.tensor_add(s0, gth[0][:, 0], gth[0][:, 1])
                nc.gpsimd.tensor_add(stg, s0, gth[1][:, 0])
                nc.vector.tensor_add(stg, stg, gth[1][:, 1])
            else:
                nc.vector.tensor_add(stg, gth[1][:, 0], gth[1][:, 1])
            stgh = stg.rearrange("g hp blk e -> g (hp blk) e")  # kh = hp*4+blk

            snorm_e = stg_pool.tile([64, H], F32)
            nc.vector.tensor_scalar_add(snorm_e, stgh[:, :, D], EPS_SLICE)
            rs = stg_pool.tile([64, H], F32)
            nc.vector.reciprocal(rs, snorm_e)
            st_sb = stg_pool.tile([64, H, D], F32)
            nc.vector.tensor_tensor(st_sb, stgh[:, :, 0:D],
                                    rs[:, :, None].to_broadcast([64, H, D]),
                                    ALU.mult)
            kv = stg_pool.tile([64, D], F32)
            nc.vector.reduce_sum(kv, st_sb.rearrange("p h d -> p d h"),
                                 axis=mybir.AxisListType.X)

            stT = stg_pool.tile([64, H, D], F32)
            for kh in range(H):
                tp = psmm.tile([64, 64], F32, tag="swtp")
                nc.tensor.transpose(tp, st_sb[:, kh, :], id32)
                nc.vector.tensor_copy(stT[:, kh, :], tp)
            kvT_p = psmm.tile([64, 64], F32, tag="swtp")
            nc.tensor.transpose(kvT_p, kv, id32)
            kvT = stg_pool.tile([64, D], F32)
            nc.vector.tensor_copy(kvT, kvT_p)

            q_ps = psacc.tile([64, H, D], F32, tag="lg2", name="q_ps")
            for kh in range(H):
                nc.tensor.matmul(q_ps[:, kh, :], stT[:, kh, :], WqT2_sb[0:64],
                                 start=(kh == 0), stop=(kh == H - 1))
            k_ps = psacc.tile([64, D], F32, tag="fx2", name="k_ps")
            nc.tensor.matmul(k_ps, kvT, WkT_sb, start=True, stop=True)
            k_sb = stg_pool.tile([64, D], F32)
            nc.vector.tensor_copy(k_sb, k_ps)
            v_ps = psacc.tile([64, D], F32, tag="fx2", name="v_ps")
            nc.tensor.matmul(v_ps, kvT, WvT_sb, start=True, stop=True)
            v_sb = stg_pool.tile([64, D], F32)
            nc.vector.tensor_copy(v_sb, v_ps)

            def rnorm(src_ps, nh, tag):
                # 1/sqrt(sum(src^2 over last dim)) via exp(-0.5*ln(.))
                sq = stg_pool.tile([64, nh, D], F32, name=f"sq_{tag}")
                nc.scalar.activation(sq, src_ps, AF.Square)
                n2 = stg_pool.tile([64, nh], F32, name=f"n2_{tag}")
                nc.vector.reduce_sum(n2, sq, axis=mybir.AxisListType.X)
                lnv = stg_pool.tile([64, nh], F32, name=f"ln_{tag}")
                nc.scalar.activation(lnv, n2, AF.Ln)
                t1 = stg_pool.tile([64, nh], F32, name=f"t1_{tag}")
                nc.scalar.activation(t1, lnv, AF.Exp, scale=-0.5)
                return t1

            rq = rnorm(q_ps, H, "q")
            rk = rnorm(k_sb[:, None, :], 1, "k")

            qn = stg_pool.tile([64, H, D], F32)
            nc.vector.tensor_tensor(qn, q_ps,
                                    rq[:, :, None].to_broadcast([64, H, D]),
                                    ALU.mult)
            kn = stg_pool.tile([64, D], F32)
            nc.vector.tensor_tensor(kn, k_sb,
                                    rk[:, 0:1].to_broadcast([64, D]), ALU.mult)

            qnT = stg_pool.tile([64, H, D], F32)
            for kh in range(H):
                tp = psmm.tile([64, 64], F32, tag="swtp")
                nc.tensor.transpose(tp, qn[:, kh, :], id32)
                nc.vector.tensor_copy(qnT[:, kh, :], tp)
            knT_p = psmm.tile([64, 64], F32, tag="swtp")
            nc.tensor.transpose(knT_p, kn, id32)
            knT = stg_pool.tile([64, D], F32)
            nc.vector.tensor_copy(knT, knT_p)

            L_ps = psacc.tile([64, H, G], F32, tag="lg2", name="L_ps")
            for kh in range(H):
                nc.tensor.matmul(L_ps[:, kh, :], qnT[:, kh, :], knT,
                                 start=(kh == 0), stop=(kh == H - 1))
            e_sb = stg_pool.tile([64, H, G], F32)
            nc.scalar.activation(e_sb, L_ps, AF.Exp, scale=attn_scale)
            aden = stg_pool.tile([64, H], F32)
            nc.vector.reduce_sum(aden, e_sb, axis=mybir.AxisListType.X)
            ra = stg_pool.tile([64, H], F32)
            nc.vector.reciprocal(ra, aden)

            LT_ps = psacc.tile([64, H, G], F32, tag="fx2", name="LT_ps")
            for kh in range(H):
                nc.tensor.matmul(LT_ps[:, kh, :], knT, qnT[:, kh, :],
                                 start=(kh == 0), stop=(kh == H - 1))
            eT_sb = stg_pool.tile([64, H, G], F32)
            nc.scalar.activation(eT_sb, LT_ps, AF.Exp, scale=attn_scale)

            av_ps = psacc.tile([64, H, D], F32, tag="lg2", name="av_ps")
            for kh in range(H):
                nc.tensor.matmul(av_ps[:, kh, :], eT_sb[:, kh, :], v_sb,
                                 start=(kh == 0), stop=(kh == H - 1))

            os_sb = stg_pool.tile([64, H, D], F32)
            nc.vector.tensor_tensor(os_sb, av_ps,
                                    ra[:, :, None].to_broadcast([64, H, D]),
                                    ALU.mult)
            rst = stg_pool.tile([64, H, D], F32)
            nc.vector.tensor_scalar_mul(rst, st_sb, res_scale)
            nc.vector.tensor_add(os_sb, os_sb, rst)

            osT = stg_pool.tile([64, H, D], BF16)
            for kh in range(H):
                tp = psmm.tile([64, 64], F32, tag="swtp")
                nc.tensor.transpose(tp, os_sb[:, kh, :], id32)
                nc.vector.tensor_copy(osT[:, kh, :], tp)

            C_sb = stg_pool.tile([128, 4, DIM], p2dt)
            for cc in range(4):
                C_ps = psmm.tile([128, DIM], F32, tag="swtp")
                for par in range(2):
                    kh = par * 4 + cc
                    h = 2 * cc + par
                    nc.tensor.matmul(C_ps[64 * par:64 * par + 64, :],
                                     osT[:, kh, :], WoT2_sb[0:64, h, :],
                                     start=True, stop=True)
                nc.vector.tensor_copy(C_sb[:, cc, :], C_ps)

            # ================= PASS 2 =================
            # out^T[f, tok] accumulated over 4 hg-chunks; C slices stationary.
            GRP = 512
            NG = NLOC // GRP  # 32
            for fb in range(2):
                for g in range(NG):
                    op = psacc.tile([128, GRP], F32,
                                    tag=["lg2", "fx2"][g % 2], name="op")
                    sw_sl = swT_store[:, :, g * GRP:(g + 1) * GRP]
                    if FP8_P2:
                        for cp in range(2):
                            nc.tensor.matmul(
                                op,
                                C_sb[:, 2 * cp:2 * cp + 2,
                                     fb * 128:(fb + 1) * 128],
                                sw_sl[:, 2 * cp:2 * cp + 2, :],
                                perf_mode=DR,
                                start=(cp == 0), stop=(cp == 1))
                    else:
                        for cc in range(4):
                            nc.tensor.matmul(
                                op,
                                C_sb[:, cc, fb * 128:(fb + 1) * 128],
                                sw_sl[:, cc, :],
                                start=(cc == 0), stop=(cc == 3))
                    if g % 2 == 0:
                        ob = obuf.tile([128, 2, GRP], F32, tag="ob")
                        nc.vector.tensor_copy(ob[:, 0, :], op)
                    else:
                        nc.scalar.copy(ob[:, 1, :], op)
                        eng = nc.sync if (g // 2) % 2 == 0 else nc.scalar
                        eng.dma_start(
                            outT_d[fb, :, (g - 1) * GRP:(g + 1) * GRP],
                            ob.rearrange("p a b -> p (a b)"))

    nc.finalize()
    return nc


def prepare(x, Wfx, bfx, Wx, bx, Wslice, bslice, temp, Wq, Wk, Wv,
            res_scale, attn_scale, Wout, bout):
    x = np.asarray(x, dtype=np.float32)
    Wfx = np.asarray(Wfx, np.float32); bfx = np.asarray(bfx, np.float32)
    Wx = np.asarray(Wx, np.float32); bx = np.asarray(bx, np.float32)
    Wslice = np.asarray(Wslice, np.float32); bslice = np.asarray(bslice, np.float32)
    temp = np.asarray(temp, np.float32).reshape(H)
    Wq = np.asarray(Wq, np.float32); Wk = np.asarray(Wk, np.float32)
    Wv = np.asarray(Wv, np.float32)
    res_scale_f = float(np.asarray(res_scale, np.float32))
    attn = np.asarray(attn_scale, np.float32).reshape(H)
    Wout = np.asarray(Wout, np.float32); bout = np.asarray(bout, np.float32)

    assert np.all(np.abs(bfx) == 0) and np.all(np.abs(bx) == 0) \
        and np.all(np.abs(bslice) == 0), "nonzero projection biases unsupported"
    assert np.ptp(attn) == 0, "non-uniform attn_scale unsupported"
    attn_f = float(attn[0])

    # folded logits weight: logits[:, h*G+g] = x @ ((Wslice @ Wx_h)/temp_h).T
    A = np.concatenate(
        [(Wslice @ Wx[h * D:(h + 1) * D, :]) / temp[h] for h in range(H)], axis=0)
    AT = np.ascontiguousarray(A.T).astype(ml_dtypes.bfloat16)
    if FP8_FX:
        WfxT = np.ascontiguousarray(Wfx.T * WSCALE).astype(NP_FP8)
    else:
        WfxT = np.ascontiguousarray(Wfx.T).astype(ml_dtypes.bfloat16)
    WoT1 = Wout.T.reshape(H, D, DIM).transpose(1, 0, 2)                # [64, 8, 256]
    WoT = np.ascontiguousarray(
        np.concatenate([WoT1, WoT1], axis=0)).astype(ml_dtypes.bfloat16)
    WqT1 = np.ascontiguousarray(Wq.T)
    WqT = np.concatenate([WqT1, WqT1], axis=0)                         # [128, 64]
    WkT = np.ascontiguousarray(Wk.T) / H
    WvT = np.ascontiguousarray(Wv.T) / H
    idbf = np.eye(128, dtype=np.float32).astype(ml_dtypes.bfloat16)
    id32 = np.eye(64, dtype=np.float32)

    key = (attn_f, res_scale_f)
    if key not in _CACHE:
        _CACHE[key] = _build(attn_f, res_scale_f)
    nc = _CACHE[key]

    in_maps = []
    for c in range(NCORES):
        b, half = c // 2, c % 2
        xs = x[b, half * NLOC:(half + 1) * NLOC, :]       # [16384, 256]
        xT = np.ascontiguousarray(xs.T.astype(ml_dtypes.bfloat16))
        in_maps.append(dict(xT=xT, AT=AT, WfxT=WfxT, idbf=idbf, id32=id32,
                            WqT=WqT, WkT=WkT, WvT=WvT, WoT=WoT))

    # fx path scale propagates linearly to the output
    descale = (1.0 / WSCALE) if FP8_FX else 1.0

    def gather(core_outs):
        out = np.empty((B, N, DIM), np.float32)
        for c in range(NCORES):
            b, half = c // 2, c % 2
            oT = np.asarray(core_outs[c]).reshape(DIM, NLOC)
            out[b, half * NLOC:(half + 1) * NLOC, :] = oT.T
        if descale != 1.0:
            out *= descale
        if np.any(bout):
            out += bout
        return out

    return dict(nc=nc, in_maps=in_maps, gather=gather)


def kernel(**inputs):
    prep = prepare(**inputs)
    global _LAST_IN_MAPS
    _LAST_IN_MAPS = prep["in_maps"]
    res = bass_utils.run_bass_kernel_spmd(
        prep["nc"], prep["in_maps"], core_ids=list(range(NCORES)))
    return prep["gather"]([res.results[c]["outT"] for c in range(NCORES)])

